# revision 3
# baseline (speedup 1.0000x reference)
"""GQA causal attention (RoPE) on 8 Trainium2 NeuronCores.

Sharding (tensor-parallel over heads, per the hint):
  core c owns q-heads {2c, 2c+1} and kv-head c//2.
  Each core computes its 2 heads' attention over the full sequence and a
  partial output projection out_c.T = wo[:, 128c:128c+128] @ att_c  (shape
  [1024, 4096]); the final all-reduce over cores is the host-side unshard.

Device-side per core:
  - Q/KV projections as fp32r matmuls (d-major layouts), RoPE applied with a
    host-side even/odd permutation folded into wq/wk so rotation pairs are
    contiguous partition blocks (lo/hi 32-blocks), sign-folded sin tile and a
    partition-block-swap SBUF->SBUF DMA.
  - scores^T blocks [sk=128, sq=512] via single K=64 matmuls (two heads hit
    disjoint PE row groups -> concurrent), causal mask added as -1e6 bias on
    the diagonal blocks, grouped exp on ScalarE from PSUM, AV matmuls with a
    ones-augmented V^T (extra column) so softmax denominators fall out of the
    same matmul (row 64 of the [65, 512] AV accumulator).
  - denominator -> K=1 broadcast matmul -> reciprocal -> per-column scale.
"""
import numpy as np
from contextlib import ExitStack

import concourse.bacc as bacc
import concourse.tile as tile
import concourse.mybir as mybir
from concourse.bass_utils import run_bass_kernel_spmd

DIM = 1024
N_HEADS = 16
N_KV = 4
HD = 64
SEQ = 4096
NCORES = 8

SQ = 512            # query-chunk (free dim of score blocks)
SK = 128            # key-chunk (partition dim of score blocks)
NQ = SEQ // SQ      # 8
NR = DIM // 128     # 8 contraction chunks for projections
GROUP = 3           # score blocks per exp instruction ([128, 1536] PSUM)
MASKVAL = -1.0e6

f32 = mybir.dt.float32
f32r = mybir.dt.float32r
FT = mybir.ActivationFunctionType

_CACHE = {}


def _emit(nc):
    xT = nc.dram_tensor("xT", [DIM, SEQ], f32r, kind="ExternalInput").ap()
    wq_l = nc.dram_tensor("wq_l", [128, DIM], f32r, kind="ExternalInput").ap()
    wkv_l = nc.dram_tensor("wkv_l", [128, DIM], f32r, kind="ExternalInput").ap()
    wo_l = nc.dram_tensor("wo_l", [128, DIM], f32r, kind="ExternalInput").ap()
    cos4_d = nc.dram_tensor("cos4", [128, SEQ], f32, kind="ExternalInput").ap()
    sin4_d = nc.dram_tensor("sin4", [128, SEQ], f32, kind="ExternalInput").ap()
    mask_d = nc.dram_tensor("mask", [128, 4 * SQ], f32, kind="ExternalInput").ap()
    id_d = nc.dram_tensor("ident", [HD, HD], f32r, kind="ExternalInput").ap()
    on_d = nc.dram_tensor("ones", [128, HD], f32r, kind="ExternalInput").ap()
    out_d = nc.dram_tensor("out", [DIM, SEQ], f32, kind="ExternalOutput").ap()

    with tile.TileContext(nc) as tc, ExitStack() as ctx:
        const = ctx.enter_context(tc.tile_pool(name="const", bufs=1))
        main = ctx.enter_context(tc.tile_pool(name="main", bufs=1))

        wq_sb = const.tile([128, DIM], f32r)
        wkv_sb = const.tile([128, DIM], f32r)
        wo_sb = const.tile([128, DIM], f32r)
        cos_sb = const.tile([128, SEQ], f32)
        sin_sb = const.tile([128, SEQ], f32)
        msk_sb = const.tile([128, 4 * SQ], f32)
        id_sb = const.tile([HD, HD], f32r)
        on_sb = const.tile([128, HD], f32r)
        nc.sync.dma_start(wq_sb[:], wq_l[:])
        nc.sync.dma_start(wkv_sb[:], wkv_l[:])
        nc.sync.dma_start(wo_sb[:], wo_l[:])
        nc.sync.dma_start(cos_sb[:], cos4_d[:])
        nc.sync.dma_start(sin_sb[:], sin4_d[:])
        nc.sync.dma_start(msk_sb[:], mask_d[:])
        nc.sync.dma_start(id_sb[:], id_d[:])
        nc.sync.dma_start(on_sb[:], on_d[:])

        qrot = main.tile([128, SEQ], f32r)      # 2 heads d-major (rope'd)
        krot = main.tile([128, SEQ], f32r)      # k duplicated in both halves
        v_sb = main.tile([HD, SEQ], f32r)       # v d-major
        vt = main.tile([128, SEQ // SK, HD + 1], f32r)  # v^T + ones column
        attS = main.tile([128, SEQ], f32r)      # stacked per-head att (j-major)
        att1 = main.tile([HD, SEQ], f32r)       # head-1 att staging (lanes 0-63)

        # ones column of vt (f32r memset unsupported -> DVE copy broadcast)
        for j in range(SEQ // SK):
            nc.vector.tensor_copy(vt[:, j, HD:HD + 1], on_sb[:, 0:1])

        # ---------------- phase 1: projections + rope ----------------
        with (
            tc.tile_pool(name="xp", bufs=4) as xp,
            tc.tile_pool(name="pp", bufs=2, space="PSUM") as pp,
            tc.tile_pool(name="rp", bufs=2) as rp,
            tc.tile_pool(name="tp", bufs=2, space="PSUM") as tp,
        ):
            for n in range(NQ):
                s0 = n * SQ
                pq = pp.tile([128, SQ], f32, tag="pq")
                pkv = pp.tile([128, SQ], f32, tag="pkv")
                for r in range(NR):
                    xt = xp.tile([128, SQ], f32r)
                    nc.sync.dma_start(xt[:], xT[128 * r:128 * (r + 1), s0:s0 + SQ])
                    nc.tensor.matmul(pq[:], wq_sb[:, 128 * r:128 * (r + 1)], xt[:],
                                     start=(r == 0), stop=(r == NR - 1))
                    nc.tensor.matmul(pkv[:], wkv_sb[:, 128 * r:128 * (r + 1)], xt[:],
                                     start=(r == 0), stop=(r == NR - 1))
                # rope q (all 128 partitions = 2 heads x (lo|hi))
                a_t = rp.tile([128, SQ], f32, tag="ta")
                c_t = rp.tile([128, SQ], f32, tag="tc")
                b_t = rp.tile([128, SQ], f32, tag="tb")
                nc.vector.tensor_mul(a_t[:], pq[:], cos_sb[:, s0:s0 + SQ])
                nc.vector.tensor_mul(c_t[:], pq[:], sin_sb[:, s0:s0 + SQ])
                nc.sync.dma_start(b_t[0:32, :], c_t[32:64, :])
                nc.sync.dma_start(b_t[32:64, :], c_t[0:32, :])
                nc.sync.dma_start(b_t[64:96, :], c_t[96:128, :])
                nc.sync.dma_start(b_t[96:128, :], c_t[64:96, :])
                nc.vector.tensor_add(qrot[:, s0:s0 + SQ], a_t[:], b_t[:])
                # rope k (rows 64:128 of pkv; v occupies rows 0:64)
                ak = rp.tile([128, SQ], f32, tag="ta")
                ck = rp.tile([128, SQ], f32, tag="tc")
                bk = rp.tile([128, SQ], f32, tag="tb")
                nc.vector.tensor_mul(ak[64:128, :], pkv[64:128, :], cos_sb[64:128, s0:s0 + SQ])
                nc.vector.tensor_mul(ck[64:128, :], pkv[64:128, :], sin_sb[64:128, s0:s0 + SQ])
                nc.sync.dma_start(bk[64:96, :], ck[96:128, :])
                nc.sync.dma_start(bk[96:128, :], ck[64:96, :])
                nc.vector.tensor_add(krot[64:128, s0:s0 + SQ], ak[64:128, :], bk[64:128, :])
                # duplicate k into rows 0:64 (head-0 row group)
                nc.sync.dma_start(krot[0:64, s0:s0 + SQ], krot[64:128, s0:s0 + SQ])
                # v copy (lanes 0-63)
                nc.vector.tensor_copy(v_sb[:, s0:s0 + SQ], pkv[0:64, :])

            # ---------------- phase 2: v transposes ----------------
            for j in range(SEQ // SK):
                pt = tp.tile([SK, HD], f32r)
                nc.tensor.transpose(pt[:], v_sb[:, SK * j:SK * (j + 1)], id_sb[:])
                nc.vector.tensor_copy(vt[:, j, 0:HD], pt[:])

        # ---------------- phase 3: attention + wo ----------------
        with (
            tc.tile_pool(name="sp", bufs=2, space="PSUM") as sp,
            tc.tile_pool(name="ap", bufs=1, space="PSUM") as ap,
            tc.tile_pool(name="ep", bufs=3) as ep,
            tc.tile_pool(name="dp", bufs=2) as dp,
            tc.tile_pool(name="rbp", bufs=2) as rbp,
            tc.tile_pool(name="op", bufs=4) as op,
        ):
            for n in range(NQ):
                s0 = n * SQ
                nsk = 4 * (n + 1)
                av = [ap.tile([HD + 1, SQ], f32, tag=f"av{h}", name=f"av{h}_{n}")
                      for h in (0, 1)]
                blocks = [(j, h) for j in range(nsk) for h in (0, 1)]
                for g0 in range(0, len(blocks), GROUP):
                    grp = blocks[g0:g0 + GROUP]
                    sc = sp.tile([128, GROUP * SQ], f32, tag="sc")
                    for i, (j, h) in enumerate(grp):
                        o = i * SQ
                        nc.tensor.matmul(
                            sc[:, o:o + SQ],
                            krot[64 * h:64 * h + 64, SK * j:SK * (j + 1)],
                            qrot[64 * h:64 * h + 64, s0:s0 + SQ],
                            start=True, stop=True,
                        )
                        delta = SK * j - s0
                        if delta >= 0:
                            w = min(SQ, delta + SK)
                            db = (delta // SK) * SQ
                            nc.vector.tensor_add(sc[:, o:o + w], sc[:, o:o + w],
                                                 msk_sb[:, db:db + w])
                    ew = len(grp) * SQ
                    et = ep.tile([128, GROUP * SQ], f32r, tag="et")
                    nc.scalar.activation(et[:, 0:ew], sc[:, 0:ew], FT.Exp, scale=0.125)
                    for i, (j, h) in enumerate(grp):
                        nc.tensor.matmul(
                            av[h][:], vt[:, j, :], et[:, i * SQ:(i + 1) * SQ],
                            start=(j == 0), stop=(j == nsk - 1),
                        )
                # normalize: denominators live in row 64 of av[h]
                for h in (0, 1):
                    d_sb = dp.tile([HD + 1, SQ], f32r, tag=f"d{h}")
                    nc.vector.tensor_copy(d_sb[HD:HD + 1, :], av[h][HD:HD + 1, :])
                    bc = sp.tile([HD, SQ], f32, tag="sc")
                    nc.tensor.matmul(bc[:], on_sb[HD:HD + 1, :], d_sb[HD:HD + 1, :],
                                     start=True, stop=True)
                    rb = rbp.tile([HD, SQ], f32, tag=f"rb{h}")
                    nc.vector.reciprocal(rb[:], bc[:])
                    dst = attS[0:HD, s0:s0 + SQ] if h == 0 else att1[:, s0:s0 + SQ]
                    nc.vector.tensor_mul(dst, av[h][0:HD, :], rb[:])
                nc.sync.dma_start(attS[64:128, s0:s0 + SQ], att1[:, s0:s0 + SQ])
                # wo for this chunk: out[128m:.., s0:..] = wo_l[:,128m:..]^T @ attS[:, s0:..]
                for m in range(8):
                    pw = sp.tile([128, SQ], f32, tag="sc")
                    nc.tensor.matmul(pw[:], wo_sb[:, 128 * m:128 * (m + 1)],
                                     attS[:, s0:s0 + SQ], start=True, stop=True)
                    ot = op.tile([128, SQ], f32)
                    nc.vector.tensor_copy(ot[:], pw[:])
                    nc.sync.dma_start(out_d[128 * m:128 * (m + 1), s0:s0 + SQ], ot[:])


def _build():
    if "nc" in _CACHE:
        return _CACHE["nc"]
    nc = bacc.Bacc("TRN2", target_bir_lowering=False, debug=False, num_devices=NCORES)
    _emit(nc)
    nc.compile()
    _CACHE["nc"] = nc
    return nc


def _host_inputs(x, freqs_cos, freqs_sin, wq, wk, wv, wo):
    x = np.asarray(x, np.float32)
    freqs_cos = np.asarray(freqs_cos, np.float32)
    freqs_sin = np.asarray(freqs_sin, np.float32)
    wq = np.asarray(wq, np.float32)
    wk = np.asarray(wk, np.float32)
    wv = np.asarray(wv, np.float32)
    wo = np.asarray(wo, np.float32)

    xT = np.ascontiguousarray(x[0].T)                       # [1024, 4096]
    cosT = freqs_cos.T                                      # [32, 4096]
    sinT = freqs_sin.T
    cos4 = np.ascontiguousarray(np.tile(cosT, (4, 1)))      # [128, 4096]
    sin4 = np.ascontiguousarray(
        np.concatenate([sinT, -sinT, sinT, -sinT], axis=0))

    # diagonal-block causal masks for delta in {0,128,256,384}
    p = np.arange(SK)[:, None]
    f = np.arange(SQ)[None, :]
    mask = np.concatenate(
        [np.where(SK * d + p <= f, 0.0, MASKVAL) for d in range(4)],
        axis=1).astype(np.float32)                          # [128, 2048]

    ident = np.eye(HD, dtype=np.float32)
    ones = np.ones((128, HD), dtype=np.float32)

    perm = np.concatenate([np.arange(0, HD, 2), np.arange(1, HD, 2)])

    def fold(w):  # [128(m), 1024(d)] -> lhsT layout [128(p), 8r*128+m]
        return np.ascontiguousarray(
            w.reshape(128, NR, 128).transpose(2, 1, 0).reshape(128, DIM))

    in_maps = []
    for c in range(NCORES):
        g = c // 2
        wq_c = wq[128 * c:128 * (c + 1)].reshape(2, HD, DIM)[:, perm, :].reshape(128, DIM)
        wk_g = wk[HD * g:HD * (g + 1)][perm]
        wv_g = wv[HD * g:HD * (g + 1)]
        wkv_c = np.concatenate([wv_g, wk_g], axis=0)        # v rows 0:64, k rows 64:128
        wo_c = np.ascontiguousarray(wo[:, 128 * c:128 * (c + 1)].T)  # [128(j), 1024(o)]
        in_maps.append({
            "xT": xT,
            "wq_l": fold(wq_c),
            "wkv_l": fold(wkv_c),
            "wo_l": wo_c,
            "cos4": cos4,
            "sin4": sin4,
            "mask": mask,
            "ident": ident,
            "ones": ones,
        })
    return in_maps


def kernel(x, freqs_cos, freqs_sin, wq, wk, wv, wo, _trace=False, _trace_kwargs=None):
    nc = _build()
    in_maps = _host_inputs(x, freqs_cos, freqs_sin, wq, wk, wv, wo)
    kw = {}
    if _trace:
        kw.update(trace=True, **(_trace_kwargs or {}))
    res = run_bass_kernel_spmd(nc, in_maps, core_ids=list(range(NCORES)), **kw)
    acc = np.zeros((DIM, SEQ), np.float32)
    for c in range(NCORES):
        acc += res.results[c]["out"]
    out = np.ascontiguousarray(acc.T).reshape(1, SEQ, DIM)
    if _trace:
        kernel._last_results = res
    return out


# revision 6
# speedup vs baseline: 1.1076x; 1.1076x over previous
"""GQA causal attention (RoPE) on 8 Trainium2 NeuronCores.

Sharding (tensor-parallel over heads, per the hint):
  core c owns q-heads {2c, 2c+1} and kv-head c//2.
  Each core computes its 2 heads' attention over the full sequence and a
  partial output projection out_c.T = wo[:, 128c:128c+128] @ att_c  (shape
  [1024, 4096]); the final all-reduce over cores is the host-side unshard.

Device-side per core (v2):
  - Projections / scores / AV / wo as bf16 matmuls (1 cycle/row on PE; fp32r
    streams at ~2 cycles/row). fp32 PSUM accumulation throughout.
  - RoPE in fp32 from PSUM with a host-side even/odd permutation folded into
    wq/wk (rotation pairs become contiguous 32-partition blocks), sign-folded
    sin tile, partition-block swap via SBUF->SBUF DMA; result rounded to bf16.
  - scores^T blocks [sk=128, sq=512]: single K=64 matmuls; the two heads hit
    disjoint PE row groups (k duplicated in both halves) -> concurrent.
    Causal masking = -1e6 adds on diagonal blocks before a grouped PSUM exp.
  - AV with ones-augmented V^T (extra column of 1s) so softmax denominators
    fall out of the same matmul; V^T built by DMA transpose (bf16).
  - Normalization deferred to a post-pass: raw AV staged to SBUF, both heads'
    denominators merged into one K=2 broadcast matmul (f32r), one reciprocal
    per chunk, per-column scale, then the wo matmuls.
"""
import numpy as np
import ml_dtypes
from contextlib import ExitStack

import concourse.bacc as bacc
import concourse.tile as tile
import concourse.mybir as mybir
from concourse.bass_utils import run_bass_kernel_spmd

DIM = 1024
N_HEADS = 16
N_KV = 4
HD = 64
SEQ = 4096
NCORES = 8

SQ = 512            # query-chunk (free dim of score blocks)
SK = 128            # key-chunk (partition dim of score blocks)
NQ = SEQ // SQ      # 8
NR = DIM // 128     # 8 contraction chunks for projections
NJ = SEQ // SK      # 32 key chunks
GROUP = 3           # score blocks per exp instruction ([128, 1536] PSUM)
MASKVAL = -1.0e6

f32 = mybir.dt.float32
f32r = mybir.dt.float32r
bf16 = mybir.dt.bfloat16
FT = mybir.ActivationFunctionType

_CACHE = {}


def _emit(nc):
    xT = nc.dram_tensor("xT", [DIM, SEQ], bf16, kind="ExternalInput").ap()
    wq_l = nc.dram_tensor("wq_l", [128, DIM], bf16, kind="ExternalInput").ap()
    wkv_l = nc.dram_tensor("wkv_l", [128, DIM], bf16, kind="ExternalInput").ap()
    wo_l = nc.dram_tensor("wo_l", [128, DIM], bf16, kind="ExternalInput").ap()
    cos4_d = nc.dram_tensor("cos4", [128, SEQ], f32, kind="ExternalInput").ap()
    sin4_d = nc.dram_tensor("sin4", [128, SEQ], f32, kind="ExternalInput").ap()
    mask_d = nc.dram_tensor("mask", [128, 4 * SQ], f32, kind="ExternalInput").ap()
    on_d = nc.dram_tensor("ones", [128, 1], bf16, kind="ExternalInput").ap()
    sel_d = nc.dram_tensor("sel2", [128, 128], f32r, kind="ExternalInput").ap()
    out_d = nc.dram_tensor("out", [DIM, SEQ], f32, kind="ExternalOutput").ap()

    with tile.TileContext(nc) as tc, ExitStack() as ctx:
        const = ctx.enter_context(tc.tile_pool(name="const", bufs=1))
        main = ctx.enter_context(tc.tile_pool(name="main", bufs=1))

        wq_sb = const.tile([128, DIM], bf16)
        wkv_sb = const.tile([128, DIM], bf16)
        wo_sb = const.tile([128, DIM], bf16)
        cos_sb = const.tile([128, SEQ], f32)
        sin_sb = const.tile([128, SEQ], f32)
        msk_sb = const.tile([128, 4 * SQ], f32)
        on_sb = const.tile([128, 1], bf16)
        sel_sb = const.tile([128, 128], f32r)
        nc.sync.dma_start(wq_sb[:], wq_l[:])
        nc.sync.dma_start(wkv_sb[:], wkv_l[:])
        nc.sync.dma_start(wo_sb[:], wo_l[:])
        nc.sync.dma_start(cos_sb[:], cos4_d[:])
        nc.sync.dma_start(sin_sb[:], sin4_d[:])
        nc.sync.dma_start(msk_sb[:], mask_d[:])
        nc.sync.dma_start(on_sb[:], on_d[:])
        nc.sync.dma_start(sel_sb[:], sel_d[:])

        qrot = main.tile([128, SEQ], bf16)      # 2 heads d-major (rope'd)
        krot = main.tile([128, SEQ], bf16)      # k duplicated in both halves
        v_sb = main.tile([HD, SEQ], bf16)       # v d-major
        vt = main.tile([128, NJ, 128], bf16)    # v^T + ones column (128-aligned slots)
        attS = main.tile([128, SEQ], bf16)      # stacked normalized att (j-major)
        att1 = main.tile([HD, SEQ], bf16)       # head-1 att staging (lanes 0-63)
        au0 = main.tile([HD + 1, SEQ], f32r)    # raw AV + denom staging, head 0
        au1 = main.tile([HD + 1, SEQ], f32r)    # head 1
        d2 = main.tile([66, SEQ], f32r)         # merged denoms (rows 64, 65)

        # ones column of vt
        for j in range(NJ):
            nc.vector.tensor_copy(vt[:, j, HD:HD + 1], on_sb[:, 0:1])

        # ---------------- phase 1: projections + rope + v ----------------
        with (
            tc.tile_pool(name="xp", bufs=4) as xp,
            tc.tile_pool(name="pp", bufs=2, space="PSUM") as pp,
            tc.tile_pool(name="rp", bufs=2) as rp,
        ):
            for n in range(NQ):
                s0 = n * SQ
                pq = pp.tile([128, SQ], f32, tag="pq")
                pkv = pp.tile([128, SQ], f32, tag="pkv")
                for r in range(NR):
                    xt = xp.tile([128, SQ], bf16)
                    nc.sync.dma_start(xt[:], xT[128 * r:128 * (r + 1), s0:s0 + SQ])
                    nc.tensor.matmul(pq[:], wq_sb[:, 128 * r:128 * (r + 1)], xt[:],
                                     start=(r == 0), stop=(r == NR - 1))
                    nc.tensor.matmul(pkv[:], wkv_sb[:, 128 * r:128 * (r + 1)], xt[:],
                                     start=(r == 0), stop=(r == NR - 1))
                # rope q (fp32 math, bf16 out)
                a_t = rp.tile([128, SQ], f32, tag="ta")
                c_t = rp.tile([128, SQ], f32, tag="tc")
                b_t = rp.tile([128, SQ], f32, tag="tb")
                nc.vector.tensor_mul(a_t[:], pq[:], cos_sb[:, s0:s0 + SQ])
                nc.vector.tensor_mul(c_t[:], pq[:], sin_sb[:, s0:s0 + SQ])
                nc.sync.dma_start(b_t[0:32, :], c_t[32:64, :])
                nc.sync.dma_start(b_t[32:64, :], c_t[0:32, :])
                nc.sync.dma_start(b_t[64:96, :], c_t[96:128, :])
                nc.sync.dma_start(b_t[96:128, :], c_t[64:96, :])
                nc.vector.tensor_add(qrot[:, s0:s0 + SQ], a_t[:], b_t[:])
                # rope k (rows 64:128 of pkv; v occupies rows 0:64)
                ak = rp.tile([128, SQ], f32, tag="ta")
                ck = rp.tile([128, SQ], f32, tag="tc")
                bk = rp.tile([128, SQ], f32, tag="tb")
                nc.vector.tensor_mul(ak[64:128, :], pkv[64:128, :], cos_sb[64:128, s0:s0 + SQ])
                nc.vector.tensor_mul(ck[64:128, :], pkv[64:128, :], sin_sb[64:128, s0:s0 + SQ])
                nc.sync.dma_start(bk[64:96, :], ck[96:128, :])
                nc.sync.dma_start(bk[96:128, :], ck[64:96, :])
                nc.vector.tensor_add(krot[64:128, s0:s0 + SQ], ak[64:128, :], bk[64:128, :])
                # duplicate k into rows 0:64 (head-0 row group)
                nc.sync.dma_start(krot[0:64, s0:s0 + SQ], krot[64:128, s0:s0 + SQ])
                # v: copy to bf16, then DMA-transpose chunks into vt
                nc.vector.tensor_copy(v_sb[:, s0:s0 + SQ], pkv[0:64, :])
                for j in range(4 * n, 4 * n + 4):
                    nc.sync.dma_start(vt[:, j, 0:HD],
                                      v_sb[:, SK * j:SK * (j + 1)], transpose=True)

        # ---------------- phase 3: attention (raw AV + denoms) ----------------
        with (
            tc.tile_pool(name="sp", bufs=2, space="PSUM") as sp,
            tc.tile_pool(name="ap", bufs=1, space="PSUM") as ap,
            tc.tile_pool(name="ep", bufs=3) as ep,
        ):
            for n in range(NQ):
                s0 = n * SQ
                nsk = 4 * (n + 1)
                av = [ap.tile([HD + 1, SQ], f32, tag=f"av{h}", name=f"av{h}_{n}")
                      for h in (0, 1)]
                blocks = [(j, h) for j in range(nsk) for h in (0, 1)]
                for g0 in range(0, len(blocks), GROUP):
                    grp = blocks[g0:g0 + GROUP]
                    sc = sp.tile([128, GROUP * SQ], f32, tag="sc")
                    for i, (j, h) in enumerate(grp):
                        o = i * SQ
                        nc.tensor.matmul(
                            sc[:, o:o + SQ],
                            krot[64 * h:64 * h + 64, SK * j:SK * (j + 1)],
                            qrot[64 * h:64 * h + 64, s0:s0 + SQ],
                            start=True, stop=True,
                        )
                        delta = SK * j - s0
                        if delta >= 0:
                            w = min(SQ, delta + SK)
                            db = (delta // SK) * SQ
                            nc.vector.tensor_add(sc[:, o:o + w], sc[:, o:o + w],
                                                 msk_sb[:, db:db + w])
                    ew = len(grp) * SQ
                    et = ep.tile([128, GROUP * SQ], bf16, tag="et")
                    nc.scalar.activation(et[:, 0:ew], sc[:, 0:ew], FT.Exp, scale=0.125)
                    for i, (j, h) in enumerate(grp):
                        nc.tensor.matmul(
                            av[h][:], vt[:, j, 0:HD + 1], et[:, i * SQ:(i + 1) * SQ],
                            start=(j == 0), stop=(j == nsk - 1),
                        )
                # stage raw AV (+denominator rows) to SBUF; free the banks fast
                nc.vector.tensor_copy(au0[:, s0:s0 + SQ], av[0][:])
                nc.vector.tensor_copy(au1[:, s0:s0 + SQ], av[1][:])

        # ---------------- phase 4: normalize + wo ----------------
        with (
            tc.tile_pool(name="bp", bufs=2, space="PSUM") as bp,
            tc.tile_pool(name="wp", bufs=4, space="PSUM") as wp,
            tc.tile_pool(name="rbp", bufs=2) as rbp,
            tc.tile_pool(name="op", bufs=4) as op,
        ):
            for n in range(NQ):
                s0 = n * SQ
                # merge denominators: row 64 <- h0, row 65 <- h1 (via DMA shift)
                nc.sync.dma_start(d2[64:65, s0:s0 + SQ], au0[HD:HD + 1, s0:s0 + SQ])
                nc.sync.dma_start(d2[65:66, s0:s0 + SQ], au1[HD:HD + 1, s0:s0 + SQ])
                bc = bp.tile([128, SQ], f32, tag="bc")
                nc.tensor.matmul(bc[:], sel_sb[64:66, :], d2[64:66, s0:s0 + SQ],
                                 start=True, stop=True)
                rb = rbp.tile([128, SQ], f32, tag="rb")
                nc.vector.reciprocal(rb[:], bc[:])
                rb1 = rbp.tile([HD, SQ], f32, tag="rb1")
                nc.sync.dma_start(rb1[:], rb[64:64 + HD, :])
                nc.vector.tensor_mul(attS[0:HD, s0:s0 + SQ],
                                     au0[0:HD, s0:s0 + SQ].bitcast(f32), rb[0:HD, :])
                nc.vector.tensor_mul(att1[:, s0:s0 + SQ],
                                     au1[0:HD, s0:s0 + SQ].bitcast(f32), rb1[:])
                nc.sync.dma_start(attS[64:128, s0:s0 + SQ], att1[:, s0:s0 + SQ])
                # wo: out[128m:.., s0:..] = wo_l[:,128m:..]^T @ attS[:, s0:..]
                for m in range(8):
                    pw = wp.tile([128, SQ], f32, tag="pw")
                    nc.tensor.matmul(pw[:], wo_sb[:, 128 * m:128 * (m + 1)],
                                     attS[:, s0:s0 + SQ], start=True, stop=True)
                    ot = op.tile([128, SQ], f32)
                    if m % 2 == 0:
                        nc.vector.tensor_copy(ot[:], pw[:])
                    else:
                        nc.scalar.copy(ot[:], pw[:])
                    nc.sync.dma_start(out_d[128 * m:128 * (m + 1), s0:s0 + SQ], ot[:])


def _build():
    if "nc" in _CACHE:
        return _CACHE["nc"]
    nc = bacc.Bacc("TRN2", target_bir_lowering=False, debug=False, num_devices=NCORES)
    _emit(nc)
    nc.compile()
    _CACHE["nc"] = nc
    return nc


def _host_inputs(x, freqs_cos, freqs_sin, wq, wk, wv, wo):
    x = np.asarray(x, np.float32)
    freqs_cos = np.asarray(freqs_cos, np.float32)
    freqs_sin = np.asarray(freqs_sin, np.float32)
    wq = np.asarray(wq, np.float32)
    wk = np.asarray(wk, np.float32)
    wv = np.asarray(wv, np.float32)
    wo = np.asarray(wo, np.float32)

    xT = np.ascontiguousarray(x[0].T).astype(ml_dtypes.bfloat16)   # [1024, 4096]
    cosT = freqs_cos.T                                             # [32, 4096]
    sinT = freqs_sin.T
    cos4 = np.ascontiguousarray(np.tile(cosT, (4, 1)))             # [128, 4096]
    sin4 = np.ascontiguousarray(
        np.concatenate([sinT, -sinT, sinT, -sinT], axis=0))

    # diagonal-block causal masks for delta in {0,128,256,384}
    p = np.arange(SK)[:, None]
    f = np.arange(SQ)[None, :]
    mask = np.concatenate(
        [np.where(SK * d + p <= f, 0.0, MASKVAL) for d in range(4)],
        axis=1).astype(np.float32)                                 # [128, 2048]

    ones = np.ones((128, 1), dtype=ml_dtypes.bfloat16)
    sel2 = np.zeros((128, 128), dtype=np.float32)
    sel2[64, 0:64] = 1.0
    sel2[65, 64:128] = 1.0

    perm = np.concatenate([np.arange(0, HD, 2), np.arange(1, HD, 2)])

    def fold(w):  # [128(m), 1024(d)] -> lhsT layout [128(p), 8r*128+m]
        return np.ascontiguousarray(
            w.reshape(128, NR, 128).transpose(2, 1, 0).reshape(128, DIM)
        ).astype(ml_dtypes.bfloat16)

    in_maps = []
    for c in range(NCORES):
        g = c // 2
        wq_c = wq[128 * c:128 * (c + 1)].reshape(2, HD, DIM)[:, perm, :].reshape(128, DIM)
        wk_g = wk[HD * g:HD * (g + 1)][perm]
        wv_g = wv[HD * g:HD * (g + 1)]
        wkv_c = np.concatenate([wv_g, wk_g], axis=0)        # v rows 0:64, k rows 64:128
        wo_c = np.ascontiguousarray(wo[:, 128 * c:128 * (c + 1)].T).astype(
            ml_dtypes.bfloat16)                              # [128(j), 1024(o)]
        in_maps.append({
            "xT": xT,
            "wq_l": fold(wq_c),
            "wkv_l": fold(wkv_c),
            "wo_l": wo_c,
            "cos4": cos4,
            "sin4": sin4,
            "mask": mask,
            "ones": ones,
            "sel2": sel2,
        })
    return in_maps


def kernel(x, freqs_cos, freqs_sin, wq, wk, wv, wo, _trace=False, _trace_kwargs=None):
    nc = _build()
    in_maps = _host_inputs(x, freqs_cos, freqs_sin, wq, wk, wv, wo)
    kw = {}
    if _trace:
        kw.update(trace=True, **(_trace_kwargs or {}))
    res = run_bass_kernel_spmd(nc, in_maps, core_ids=list(range(NCORES)), **kw)
    acc = np.zeros((DIM, SEQ), np.float32)
    for c in range(NCORES):
        acc += res.results[c]["out"]
    out = np.ascontiguousarray(acc.T).reshape(1, SEQ, DIM)
    if _trace:
        kernel._last_results = res
    return out


# revision 7
# speedup vs baseline: 1.1967x; 1.0805x over previous
"""GQA causal attention (RoPE) on 8 Trainium2 NeuronCores.

Sharding (tensor-parallel over heads, per the hint):
  core c owns q-heads {2c, 2c+1} and kv-head c//2.
  Each core computes its 2 heads' attention over the full sequence and a
  partial output projection out_c.T = wo[:, 128c:128c+128] @ att_c  (shape
  [1024, 4096]); the final all-reduce over cores is the host-side unshard.

Device-side per core (v3 — single fused loop over 512-row query chunks):
  proj(n) -> rope(n) -> v^T(n) -> attention(n) -> normalize+wo(n), all sharing
  one 8-bank PSUM budget so chunk n+1's projections gap-fill the PE while
  chunk n's softmax runs (keeps the PE HAM-warm).

  - All matmuls bf16 (1 cycle/row on PE; fp32r streams at ~2 cycles/row),
    fp32 PSUM accumulation.
  - RoPE in fp32 from PSUM: host-side even/odd permutation folded into wq/wk
    (rotation pairs become contiguous 32-partition blocks), sign-folded sin
    tile, partition-block swap via SBUF->SBUF DMA; result rounded to bf16.
  - scores^T blocks [sk=128, sq=512]: single K=64 matmuls; the two heads hit
    disjoint PE row groups (k duplicated in both halves) -> concurrent.
    Causal masking = -1e6 adds on diagonal blocks before a grouped PSUM exp.
  - AV with ones-augmented V^T (extra column of 1s) so softmax denominators
    fall out of the same matmul; V^T via PE transpose-mode (bf16).
  - Normalization: raw AV staged to SBUF, both heads' denominators merged into
    one K=2 broadcast matmul (f32r), one reciprocal per chunk, per-column
    scale, then the 8 wo matmuls for the chunk.
"""
import numpy as np
import ml_dtypes
from contextlib import ExitStack

import concourse.bacc as bacc
import concourse.tile as tile
import concourse.mybir as mybir
from concourse.bass_utils import run_bass_kernel_spmd

DIM = 1024
N_HEADS = 16
N_KV = 4
HD = 64
SEQ = 4096
NCORES = 8

SQ = 512            # query-chunk (free dim of score blocks)
SK = 128            # key-chunk (partition dim of score blocks)
NQ = SEQ // SQ      # 8
NR = DIM // 128     # 8 contraction chunks for projections
NJ = SEQ // SK      # 32 key chunks
GROUP = 2           # score blocks per exp instruction ([128, 1024] PSUM)
MASKVAL = -1.0e6

f32 = mybir.dt.float32
f32r = mybir.dt.float32r
bf16 = mybir.dt.bfloat16
FT = mybir.ActivationFunctionType

_CACHE = {}


def _emit(nc):
    xT = nc.dram_tensor("xT", [DIM, SEQ], bf16, kind="ExternalInput").ap()
    wq_l = nc.dram_tensor("wq_l", [128, DIM], bf16, kind="ExternalInput").ap()
    wkv_l = nc.dram_tensor("wkv_l", [128, DIM], bf16, kind="ExternalInput").ap()
    wo_l = nc.dram_tensor("wo_l", [128, DIM], bf16, kind="ExternalInput").ap()
    cos4_d = nc.dram_tensor("cos4", [128, SEQ], f32, kind="ExternalInput").ap()
    sin4_d = nc.dram_tensor("sin4", [128, SEQ], f32, kind="ExternalInput").ap()
    mask_d = nc.dram_tensor("mask", [128, 4 * SQ], f32, kind="ExternalInput").ap()
    on_d = nc.dram_tensor("ones", [128, 1], bf16, kind="ExternalInput").ap()
    id_d = nc.dram_tensor("ident", [HD, HD], bf16, kind="ExternalInput").ap()
    sel_d = nc.dram_tensor("sel2", [128, 128], f32r, kind="ExternalInput").ap()
    out_d = nc.dram_tensor("out", [DIM, SEQ], f32, kind="ExternalOutput").ap()

    with tile.TileContext(nc) as tc, ExitStack() as ctx:
        const = ctx.enter_context(tc.tile_pool(name="const", bufs=1))
        main = ctx.enter_context(tc.tile_pool(name="main", bufs=1))

        wq_sb = const.tile([128, DIM], bf16)
        wkv_sb = const.tile([128, DIM], bf16)
        wo_sb = const.tile([128, DIM], bf16)
        cos_sb = const.tile([128, SEQ], f32)
        sin_sb = const.tile([128, SEQ], f32)
        msk_sb = const.tile([128, 4 * SQ], f32)
        on_sb = const.tile([128, 1], bf16)
        id_sb = const.tile([HD, HD], bf16)
        sel_sb = const.tile([128, 128], f32r)
        nc.sync.dma_start(wq_sb[:], wq_l[:])
        nc.sync.dma_start(wkv_sb[:], wkv_l[:])
        nc.sync.dma_start(wo_sb[:], wo_l[:])
        nc.sync.dma_start(on_sb[:], on_d[:])
        nc.sync.dma_start(id_sb[:], id_d[:])
        nc.sync.dma_start(sel_sb[:], sel_d[:])
        nc.sync.dma_start(msk_sb[:], mask_d[:])
        # chunked trig loads so chunk-0 rope doesn't wait on the full 4MB
        for n in range(NQ):
            nc.sync.dma_start(cos_sb[:, n * SQ:(n + 1) * SQ], cos4_d[:, n * SQ:(n + 1) * SQ])
            nc.sync.dma_start(sin_sb[:, n * SQ:(n + 1) * SQ], sin4_d[:, n * SQ:(n + 1) * SQ])

        qrot = main.tile([128, SEQ], bf16)      # 2 heads d-major (rope'd)
        krot = main.tile([128, SEQ], bf16)      # k duplicated in both halves
        v_sb = main.tile([HD, SEQ], bf16)       # v d-major
        vt = main.tile([128, NJ, 128], bf16)    # v^T + ones column (aligned slots)
        attS = main.tile([128, SEQ], bf16)      # stacked normalized att (j-major)
        att1 = main.tile([HD, SEQ], bf16)       # head-1 att staging (lanes 0-63)
        au0 = main.tile([HD + 1, SEQ], f32r)    # raw AV + denom staging, head 0
        au1 = main.tile([HD + 1, SEQ], f32r)    # head 1
        d2 = main.tile([66, SEQ], f32r)         # merged denoms (rows 64, 65)

        for j in range(NJ):
            nc.vector.tensor_copy(vt[:, j, HD:HD + 1], on_sb[:, 0:1])

        with (
            tc.tile_pool(name="xp", bufs=4) as xp,
            tc.tile_pool(name="pp", bufs=1, space="PSUM") as pp,
            tc.tile_pool(name="rp", bufs=2) as rp,
            tc.tile_pool(name="sp", bufs=2, space="PSUM") as sp,
            tc.tile_pool(name="ap", bufs=1, space="PSUM") as ap,
            tc.tile_pool(name="ep", bufs=3) as ep,
            tc.tile_pool(name="rbp", bufs=2) as rbp,
            tc.tile_pool(name="op", bufs=4) as op,
        ):
            for n in range(NQ):
                s0 = n * SQ
                # ---- projections ----
                pq = pp.tile([128, SQ], f32, tag="pq")
                pkv = pp.tile([128, SQ], f32, tag="pkv")
                for r in range(NR):
                    xt = xp.tile([128, SQ], bf16)
                    nc.sync.dma_start(xt[:], xT[128 * r:128 * (r + 1), s0:s0 + SQ])
                    nc.tensor.matmul(pq[:], wq_sb[:, 128 * r:128 * (r + 1)], xt[:],
                                     start=(r == 0), stop=(r == NR - 1))
                    nc.tensor.matmul(pkv[:], wkv_sb[:, 128 * r:128 * (r + 1)], xt[:],
                                     start=(r == 0), stop=(r == NR - 1))
                # ---- rope q ----
                a_t = rp.tile([128, SQ], f32, tag="ta")
                c_t = rp.tile([128, SQ], f32, tag="tc")
                b_t = rp.tile([128, SQ], f32, tag="tb")
                nc.vector.tensor_mul(a_t[:], pq[:], cos_sb[:, s0:s0 + SQ])
                nc.vector.tensor_mul(c_t[:], pq[:], sin_sb[:, s0:s0 + SQ])
                nc.sync.dma_start(b_t[0:32, :], c_t[32:64, :])
                nc.sync.dma_start(b_t[32:64, :], c_t[0:32, :])
                nc.sync.dma_start(b_t[64:96, :], c_t[96:128, :])
                nc.sync.dma_start(b_t[96:128, :], c_t[64:96, :])
                nc.vector.tensor_add(qrot[:, s0:s0 + SQ], a_t[:], b_t[:])
                # ---- rope k (rows 64:128; v occupies rows 0:64) ----
                ak = rp.tile([128, SQ], f32, tag="ta")
                ck = rp.tile([128, SQ], f32, tag="tc")
                bk = rp.tile([128, SQ], f32, tag="tb")
                nc.vector.tensor_mul(ak[64:128, :], pkv[64:128, :], cos_sb[64:128, s0:s0 + SQ])
                nc.vector.tensor_mul(ck[64:128, :], pkv[64:128, :], sin_sb[64:128, s0:s0 + SQ])
                nc.sync.dma_start(bk[64:96, :], ck[96:128, :])
                nc.sync.dma_start(bk[96:128, :], ck[64:96, :])
                nc.vector.tensor_add(krot[64:128, s0:s0 + SQ], ak[64:128, :], bk[64:128, :])
                nc.sync.dma_start(krot[0:64, s0:s0 + SQ], krot[64:128, s0:s0 + SQ])
                # ---- v -> bf16, PE transpose into vt ----
                nc.vector.tensor_copy(v_sb[:, s0:s0 + SQ], pkv[0:64, :])
                for j in range(4 * n, 4 * n + 4):
                    pt = sp.tile([SK, HD], bf16, tag="sc", name=f"pt_{j}")
                    nc.tensor.transpose(pt[:], v_sb[:, SK * j:SK * (j + 1)], id_sb[:])
                    nc.vector.tensor_copy(vt[:, j, 0:HD], pt[:])

                # ---- attention ----
                nsk = 4 * (n + 1)
                av = [ap.tile([HD + 1, SQ], f32, tag=f"av{h}", name=f"av{h}_{n}")
                      for h in (0, 1)]
                blocks = [(j, h) for j in range(nsk) for h in (0, 1)]
                for g0 in range(0, len(blocks), GROUP):
                    grp = blocks[g0:g0 + GROUP]
                    sc = sp.tile([128, GROUP * SQ], f32, tag="sc")
                    for i, (j, h) in enumerate(grp):
                        o = i * SQ
                        nc.tensor.matmul(
                            sc[:, o:o + SQ],
                            krot[64 * h:64 * h + 64, SK * j:SK * (j + 1)],
                            qrot[64 * h:64 * h + 64, s0:s0 + SQ],
                            start=True, stop=True,
                        )
                        delta = SK * j - s0
                        if delta >= 0:
                            w = min(SQ, delta + SK)
                            db = (delta // SK) * SQ
                            nc.vector.tensor_add(sc[:, o:o + w], sc[:, o:o + w],
                                                 msk_sb[:, db:db + w])
                    ew = len(grp) * SQ
                    et = ep.tile([128, GROUP * SQ], bf16, tag="et")
                    nc.scalar.activation(et[:, 0:ew], sc[:, 0:ew], FT.Exp, scale=0.125)
                    for i, (j, h) in enumerate(grp):
                        nc.tensor.matmul(
                            av[h][:], vt[:, j, 0:HD + 1], et[:, i * SQ:(i + 1) * SQ],
                            start=(j == 0), stop=(j == nsk - 1),
                        )
                # ---- stage raw AV, free banks ----
                nc.vector.tensor_copy(au0[:, s0:s0 + SQ], av[0][:])
                nc.vector.tensor_copy(au1[:, s0:s0 + SQ], av[1][:])
                # ---- normalize ----
                nc.sync.dma_start(d2[64:65, s0:s0 + SQ], au0[HD:HD + 1, s0:s0 + SQ])
                nc.sync.dma_start(d2[65:66, s0:s0 + SQ], au1[HD:HD + 1, s0:s0 + SQ])
                bc = sp.tile([128, SQ], f32, tag="sc", name=f"bc_{n}")
                nc.tensor.matmul(bc[:], sel_sb[64:66, :], d2[64:66, s0:s0 + SQ],
                                 start=True, stop=True)
                rb = rbp.tile([128, SQ], f32, tag="rb")
                nc.vector.reciprocal(rb[:], bc[:])
                rb1 = rbp.tile([HD, SQ], f32, tag="rb1")
                nc.sync.dma_start(rb1[:], rb[64:64 + HD, :])
                nc.vector.tensor_mul(attS[0:HD, s0:s0 + SQ],
                                     au0[0:HD, s0:s0 + SQ].bitcast(f32), rb[0:HD, :])
                nc.vector.tensor_mul(att1[:, s0:s0 + SQ],
                                     au1[0:HD, s0:s0 + SQ].bitcast(f32), rb1[:])
                nc.sync.dma_start(attS[64:128, s0:s0 + SQ], att1[:, s0:s0 + SQ])
                # ---- wo ----
                for m in range(8):
                    pw = sp.tile([128, SQ], f32, tag="sc", name=f"pw_{n}_{m}")
                    nc.tensor.matmul(pw[:], wo_sb[:, 128 * m:128 * (m + 1)],
                                     attS[:, s0:s0 + SQ], start=True, stop=True)
                    ot = op.tile([128, SQ], f32)
                    if m % 2 == 0:
                        nc.vector.tensor_copy(ot[:], pw[:])
                    else:
                        nc.scalar.copy(ot[:], pw[:])
                    nc.sync.dma_start(out_d[128 * m:128 * (m + 1), s0:s0 + SQ], ot[:])


def _build():
    if "nc" in _CACHE:
        return _CACHE["nc"]
    nc = bacc.Bacc("TRN2", target_bir_lowering=False, debug=False, num_devices=NCORES)
    _emit(nc)
    nc.compile()
    _CACHE["nc"] = nc
    return nc


def _host_inputs(x, freqs_cos, freqs_sin, wq, wk, wv, wo):
    x = np.asarray(x, np.float32)
    freqs_cos = np.asarray(freqs_cos, np.float32)
    freqs_sin = np.asarray(freqs_sin, np.float32)
    wq = np.asarray(wq, np.float32)
    wk = np.asarray(wk, np.float32)
    wv = np.asarray(wv, np.float32)
    wo = np.asarray(wo, np.float32)

    xT = np.ascontiguousarray(x[0].T).astype(ml_dtypes.bfloat16)   # [1024, 4096]
    cosT = freqs_cos.T                                             # [32, 4096]
    sinT = freqs_sin.T
    cos4 = np.ascontiguousarray(np.tile(cosT, (4, 1)))             # [128, 4096]
    sin4 = np.ascontiguousarray(
        np.concatenate([sinT, -sinT, sinT, -sinT], axis=0))

    # diagonal-block causal masks for delta in {0,128,256,384}
    p = np.arange(SK)[:, None]
    f = np.arange(SQ)[None, :]
    mask = np.concatenate(
        [np.where(SK * d + p <= f, 0.0, MASKVAL) for d in range(4)],
        axis=1).astype(np.float32)                                 # [128, 2048]

    ones = np.ones((128, 1), dtype=ml_dtypes.bfloat16)
    ident = np.eye(HD, dtype=ml_dtypes.bfloat16)
    sel2 = np.zeros((128, 128), dtype=np.float32)
    sel2[64, 0:64] = 1.0
    sel2[65, 64:128] = 1.0

    perm = np.concatenate([np.arange(0, HD, 2), np.arange(1, HD, 2)])

    def fold(w):  # [128(m), 1024(d)] -> lhsT layout [128(p), 8r*128+m]
        return np.ascontiguousarray(
            w.reshape(128, NR, 128).transpose(2, 1, 0).reshape(128, DIM)
        ).astype(ml_dtypes.bfloat16)

    in_maps = []
    for c in range(NCORES):
        g = c // 2
        wq_c = wq[128 * c:128 * (c + 1)].reshape(2, HD, DIM)[:, perm, :].reshape(128, DIM)
        wk_g = wk[HD * g:HD * (g + 1)][perm]
        wv_g = wv[HD * g:HD * (g + 1)]
        wkv_c = np.concatenate([wv_g, wk_g], axis=0)        # v rows 0:64, k rows 64:128
        wo_c = np.ascontiguousarray(wo[:, 128 * c:128 * (c + 1)].T).astype(
            ml_dtypes.bfloat16)                              # [128(j), 1024(o)]
        in_maps.append({
            "xT": xT,
            "wq_l": fold(wq_c),
            "wkv_l": fold(wkv_c),
            "wo_l": wo_c,
            "cos4": cos4,
            "sin4": sin4,
            "mask": mask,
            "ones": ones,
            "ident": ident,
            "sel2": sel2,
        })
    return in_maps


def kernel(x, freqs_cos, freqs_sin, wq, wk, wv, wo, _trace=False, _trace_kwargs=None):
    nc = _build()
    in_maps = _host_inputs(x, freqs_cos, freqs_sin, wq, wk, wv, wo)
    kw = {}
    if _trace:
        kw.update(trace=True, **(_trace_kwargs or {}))
    res = run_bass_kernel_spmd(nc, in_maps, core_ids=list(range(NCORES)), **kw)
    acc = np.zeros((DIM, SEQ), np.float32)
    for c in range(NCORES):
        acc += res.results[c]["out"]
    out = np.ascontiguousarray(acc.T).reshape(1, SEQ, DIM)
    if _trace:
        kernel._last_results = res
    return out


# revision 8
# speedup vs baseline: 1.2413x; 1.0373x over previous
"""GQA causal attention (RoPE) on 8 Trainium2 NeuronCores.

Sharding (tensor-parallel over heads, per the hint):
  core c owns q-heads {2c, 2c+1} and kv-head c//2.
  Each core computes its 2 heads' attention over the full sequence and a
  partial output projection out_c.T = wo[:, 128c:128c+128] @ att_c  (shape
  [1024, 4096]); the final all-reduce over cores is the host-side unshard.

Device-side per core (v4 — fused + software-pipelined over 512-row chunks):
  loop n: proj(n) -> rope(n) -> v^T(n) -> normalize+wo(n-1) -> attention(n).
  The chunk-(n-1) normalization (DVE reciprocal chain) overlaps chunk-n PE
  work, so the PE stream never waits on it and stays HAM-warm.

  - All matmuls bf16 (1 cycle/row; fp32r streams at ~2 cycles/row), fp32 PSUM.
  - RoPE in fp32 from PSUM: host-side even/odd permutation folded into wq/wk,
    sign-folded sin tile, partition-block swap via SBUF->SBUF DMA; bf16 out.
  - scores^T blocks [sk=128, sq=512]: single K=64 matmuls; the two heads hit
    disjoint PE row groups (k duplicated in both halves) -> concurrent.
  - Causal masks accumulated on the PE (identity matmul, -1e6 bf16 additive
    masks) before a grouped PSUM exp on ScalarE.
  - AV with ones-augmented V^T so softmax denominators fall out of the same
    matmul; V^T via PE transpose-mode, evacuated by ScalarE copies.
  - Normalization: raw AV staged to SBUF (f32r), both heads' denominators
    merged into one K=2 broadcast matmul, one reciprocal per chunk,
    per-column scale, then the 8 wo matmuls for the chunk.
"""
import numpy as np
import ml_dtypes
from contextlib import ExitStack

import concourse.bacc as bacc
import concourse.tile as tile
import concourse.mybir as mybir
from concourse.bass_utils import run_bass_kernel_spmd

DIM = 1024
N_HEADS = 16
N_KV = 4
HD = 64
SEQ = 4096
NCORES = 8

SQ = 512            # query-chunk (free dim of score blocks)
SK = 128            # key-chunk (partition dim of score blocks)
NQ = SEQ // SQ      # 8
NR = DIM // 128     # 8 contraction chunks for projections
NJ = SEQ // SK      # 32 key chunks
GROUP = 2           # score blocks per exp instruction ([128, 1024] PSUM)
MASKVAL = -1.0e6

f32 = mybir.dt.float32
f32r = mybir.dt.float32r
bf16 = mybir.dt.bfloat16
FT = mybir.ActivationFunctionType

_CACHE = {}


def _emit(nc):
    xT = nc.dram_tensor("xT", [DIM, SEQ], bf16, kind="ExternalInput").ap()
    wq_l = nc.dram_tensor("wq_l", [128, DIM], bf16, kind="ExternalInput").ap()
    wkv_l = nc.dram_tensor("wkv_l", [128, DIM], bf16, kind="ExternalInput").ap()
    wo_l = nc.dram_tensor("wo_l", [128, DIM], bf16, kind="ExternalInput").ap()
    cos4_d = nc.dram_tensor("cos4", [128, SEQ], f32, kind="ExternalInput").ap()
    sin4_d = nc.dram_tensor("sin4", [128, SEQ], f32, kind="ExternalInput").ap()
    mask_d = nc.dram_tensor("mask", [128, 4 * SQ], bf16, kind="ExternalInput").ap()
    on_d = nc.dram_tensor("ones32", [128, NJ], bf16, kind="ExternalInput").ap()
    id_d = nc.dram_tensor("ident", [128, 128], bf16, kind="ExternalInput").ap()
    sel_d = nc.dram_tensor("sel2", [128, 128], f32r, kind="ExternalInput").ap()
    out_d = nc.dram_tensor("out", [DIM, SEQ], f32, kind="ExternalOutput").ap()

    with tile.TileContext(nc) as tc, ExitStack() as ctx:
        const = ctx.enter_context(tc.tile_pool(name="const", bufs=1))
        main = ctx.enter_context(tc.tile_pool(name="main", bufs=1))

        wq_sb = const.tile([128, DIM], bf16)
        wkv_sb = const.tile([128, DIM], bf16)
        wo_sb = const.tile([128, DIM], bf16)
        cos_sb = const.tile([128, SEQ], f32)
        sin_sb = const.tile([128, SEQ], f32)
        msk_sb = const.tile([128, 4 * SQ], bf16)
        id_sb = const.tile([128, 128], bf16)
        sel_sb = const.tile([128, 128], f32r)
        nc.sync.dma_start(wq_sb[:], wq_l[:])
        nc.sync.dma_start(wkv_sb[:], wkv_l[:])
        nc.sync.dma_start(wo_sb[:], wo_l[:])
        nc.sync.dma_start(id_sb[:], id_d[:])
        nc.sync.dma_start(sel_sb[:], sel_d[:])
        nc.sync.dma_start(msk_sb[:], mask_d[:])

        qrot = main.tile([128, SEQ], bf16)      # 2 heads d-major (rope'd)
        krot = main.tile([128, SEQ], bf16)      # k duplicated in both halves
        v_sb = main.tile([HD, SEQ], bf16)       # v d-major
        vt = main.tile([128, NJ, 128], bf16)    # v^T + ones column (aligned slots)
        attS = main.tile([128, SEQ], bf16)      # stacked normalized att (j-major)
        att1 = main.tile([HD, SEQ], bf16)       # head-1 att staging (lanes 0-63)
        au0 = main.tile([HD + 1, SEQ], f32r)    # raw AV + denom staging, head 0
        au1 = main.tile([HD + 1, SEQ], f32r)    # head 1
        d2 = main.tile([66, SEQ], f32r)         # merged denoms (rows 64, 65)

        # ones column of vt via one DMA
        nc.sync.dma_start(vt[:, :, HD:HD + 1], on_d[:])
        # chunked trig loads so chunk-0 rope doesn't wait on the full 4MB
        for n in range(NQ):
            nc.sync.dma_start(cos_sb[:, n * SQ:(n + 1) * SQ], cos4_d[:, n * SQ:(n + 1) * SQ])
            nc.sync.dma_start(sin_sb[:, n * SQ:(n + 1) * SQ], sin4_d[:, n * SQ:(n + 1) * SQ])

        with (
            tc.tile_pool(name="xp", bufs=4) as xp,
            tc.tile_pool(name="pp", bufs=1, space="PSUM") as pp,
            tc.tile_pool(name="rp", bufs=2) as rp,
            tc.tile_pool(name="sp", bufs=2, space="PSUM") as sp,
            tc.tile_pool(name="ap", bufs=1, space="PSUM") as ap,
            tc.tile_pool(name="ep", bufs=3) as ep,
            tc.tile_pool(name="rbp", bufs=2) as rbp,
            tc.tile_pool(name="op", bufs=4) as op,
        ):
            def endgame(k):
                sk0 = k * SQ
                nc.sync.dma_start(d2[64:65, sk0:sk0 + SQ], au0[HD:HD + 1, sk0:sk0 + SQ])
                nc.sync.dma_start(d2[65:66, sk0:sk0 + SQ], au1[HD:HD + 1, sk0:sk0 + SQ])
                bc = sp.tile([128, SQ], f32, tag="sc", name=f"bc_{k}")
                nc.tensor.matmul(bc[:], sel_sb[64:66, :], d2[64:66, sk0:sk0 + SQ],
                                 start=True, stop=True)
                rb = rbp.tile([128, SQ], f32, tag="rb")
                nc.vector.reciprocal(rb[:], bc[:])
                rb1 = rbp.tile([HD, SQ], f32, tag="rb1")
                nc.sync.dma_start(rb1[:], rb[64:64 + HD, :])
                nc.vector.tensor_mul(attS[0:HD, sk0:sk0 + SQ],
                                     au0[0:HD, sk0:sk0 + SQ].bitcast(f32), rb[0:HD, :])
                nc.vector.tensor_mul(att1[:, sk0:sk0 + SQ],
                                     au1[0:HD, sk0:sk0 + SQ].bitcast(f32), rb1[:])
                nc.sync.dma_start(attS[64:128, sk0:sk0 + SQ], att1[:, sk0:sk0 + SQ])
                for m in range(8):
                    pw = sp.tile([128, SQ], f32, tag="sc", name=f"pw_{k}_{m}")
                    nc.tensor.matmul(pw[:], wo_sb[:, 128 * m:128 * (m + 1)],
                                     attS[:, sk0:sk0 + SQ], start=True, stop=True)
                    ot = op.tile([128, SQ], f32)
                    if m % 2 == 0:
                        nc.vector.tensor_copy(ot[:], pw[:])
                    else:
                        nc.scalar.copy(ot[:], pw[:])
                    nc.sync.dma_start(out_d[128 * m:128 * (m + 1), sk0:sk0 + SQ], ot[:])

            for n in range(NQ):
                s0 = n * SQ
                # ---- projections ----
                pq = pp.tile([128, SQ], f32, tag="pq")
                pkv = pp.tile([128, SQ], f32, tag="pkv")
                for r in range(NR):
                    xt = xp.tile([128, SQ], bf16)
                    nc.sync.dma_start(xt[:], xT[128 * r:128 * (r + 1), s0:s0 + SQ])
                    nc.tensor.matmul(pq[:], wq_sb[:, 128 * r:128 * (r + 1)], xt[:],
                                     start=(r == 0), stop=(r == NR - 1))
                    nc.tensor.matmul(pkv[:], wkv_sb[:, 128 * r:128 * (r + 1)], xt[:],
                                     start=(r == 0), stop=(r == NR - 1))
                # ---- rope q ----
                a_t = rp.tile([128, SQ], f32, tag="ta")
                c_t = rp.tile([128, SQ], f32, tag="tc")
                b_t = rp.tile([128, SQ], f32, tag="tb")
                nc.vector.tensor_mul(a_t[:], pq[:], cos_sb[:, s0:s0 + SQ])
                nc.vector.tensor_mul(c_t[:], pq[:], sin_sb[:, s0:s0 + SQ])
                nc.sync.dma_start(b_t[0:32, :], c_t[32:64, :])
                nc.sync.dma_start(b_t[32:64, :], c_t[0:32, :])
                nc.sync.dma_start(b_t[64:96, :], c_t[96:128, :])
                nc.sync.dma_start(b_t[96:128, :], c_t[64:96, :])
                nc.vector.tensor_add(qrot[:, s0:s0 + SQ], a_t[:], b_t[:])
                # ---- rope k (rows 64:128; v occupies rows 0:64) ----
                ak = rp.tile([128, SQ], f32, tag="ta")
                ck = rp.tile([128, SQ], f32, tag="tc")
                bk = rp.tile([128, SQ], f32, tag="tb")
                nc.vector.tensor_mul(ak[64:128, :], pkv[64:128, :], cos_sb[64:128, s0:s0 + SQ])
                nc.vector.tensor_mul(ck[64:128, :], pkv[64:128, :], sin_sb[64:128, s0:s0 + SQ])
                nc.sync.dma_start(bk[64:96, :], ck[96:128, :])
                nc.sync.dma_start(bk[96:128, :], ck[64:96, :])
                nc.vector.tensor_add(krot[64:128, s0:s0 + SQ], ak[64:128, :], bk[64:128, :])
                nc.sync.dma_start(krot[0:64, s0:s0 + SQ], krot[64:128, s0:s0 + SQ])
                # ---- v -> bf16, PE transpose into vt (ScalarE evacuates) ----
                nc.vector.tensor_copy(v_sb[:, s0:s0 + SQ], pkv[0:64, :])
                for j in range(4 * n, 4 * n + 4):
                    pt = sp.tile([SK, HD], bf16, tag="sc", name=f"pt_{j}")
                    nc.tensor.transpose(pt[:], v_sb[:, SK * j:SK * (j + 1)],
                                        id_sb[0:HD, 0:HD])
                    nc.scalar.copy(vt[:, j, 0:HD], pt[:])

                # ---- previous chunk's normalization + wo (overlaps PE work) ----
                if n > 0:
                    endgame(n - 1)

                # ---- attention ----
                nsk = 4 * (n + 1)
                av = [ap.tile([HD + 1, SQ], f32, tag=f"av{h}", name=f"av{h}_{n}")
                      for h in (0, 1)]
                blocks = [(j, h) for j in range(nsk) for h in (0, 1)]
                for g0 in range(0, len(blocks), GROUP):
                    grp = blocks[g0:g0 + GROUP]
                    sc = sp.tile([128, GROUP * SQ], f32, tag="sc")
                    for i, (j, h) in enumerate(grp):
                        o = i * SQ
                        delta = SK * j - s0
                        diag = delta >= 0
                        nc.tensor.matmul(
                            sc[:, o:o + SQ],
                            krot[64 * h:64 * h + 64, SK * j:SK * (j + 1)],
                            qrot[64 * h:64 * h + 64, s0:s0 + SQ],
                            start=True, stop=not diag,
                        )
                        if diag:
                            w = min(SQ, delta + SK)
                            db = (delta // SK) * SQ
                            nc.tensor.matmul(sc[:, o:o + w], id_sb[:],
                                             msk_sb[:, db:db + w],
                                             start=False, stop=True)
                    ew = len(grp) * SQ
                    et = ep.tile([128, GROUP * SQ], bf16, tag="et")
                    nc.scalar.activation(et[:, 0:ew], sc[:, 0:ew], FT.Exp, scale=0.125)
                    for i, (j, h) in enumerate(grp):
                        nc.tensor.matmul(
                            av[h][:], vt[:, j, 0:HD + 1], et[:, i * SQ:(i + 1) * SQ],
                            start=(j == 0), stop=(j == nsk - 1),
                        )
                # ---- stage raw AV, free banks ----
                nc.vector.tensor_copy(au0[:, s0:s0 + SQ], av[0][:])
                nc.vector.tensor_copy(au1[:, s0:s0 + SQ], av[1][:])

            endgame(NQ - 1)


def _build():
    if "nc" in _CACHE:
        return _CACHE["nc"]
    nc = bacc.Bacc("TRN2", target_bir_lowering=False, debug=False, num_devices=NCORES)
    _emit(nc)
    nc.compile()
    _CACHE["nc"] = nc
    return nc


def _host_inputs(x, freqs_cos, freqs_sin, wq, wk, wv, wo):
    x = np.asarray(x, np.float32)
    freqs_cos = np.asarray(freqs_cos, np.float32)
    freqs_sin = np.asarray(freqs_sin, np.float32)
    wq = np.asarray(wq, np.float32)
    wk = np.asarray(wk, np.float32)
    wv = np.asarray(wv, np.float32)
    wo = np.asarray(wo, np.float32)

    xT = np.ascontiguousarray(x[0].T).astype(ml_dtypes.bfloat16)   # [1024, 4096]
    cosT = freqs_cos.T                                             # [32, 4096]
    sinT = freqs_sin.T
    cos4 = np.ascontiguousarray(np.tile(cosT, (4, 1)))             # [128, 4096]
    sin4 = np.ascontiguousarray(
        np.concatenate([sinT, -sinT, sinT, -sinT], axis=0))

    # diagonal-block causal masks for delta in {0,128,256,384}
    p = np.arange(SK)[:, None]
    f = np.arange(SQ)[None, :]
    mask = np.concatenate(
        [np.where(SK * d + p <= f, 0.0, MASKVAL) for d in range(4)],
        axis=1).astype(ml_dtypes.bfloat16)                         # [128, 2048]

    ones32 = np.ones((128, NJ), dtype=ml_dtypes.bfloat16)
    ident = np.eye(128, dtype=ml_dtypes.bfloat16)
    sel2 = np.zeros((128, 128), dtype=np.float32)
    sel2[64, 0:64] = 1.0
    sel2[65, 64:128] = 1.0

    perm = np.concatenate([np.arange(0, HD, 2), np.arange(1, HD, 2)])

    def fold(w):  # [128(m), 1024(d)] -> lhsT layout [128(p), 8r*128+m]
        return np.ascontiguousarray(
            w.reshape(128, NR, 128).transpose(2, 1, 0).reshape(128, DIM)
        ).astype(ml_dtypes.bfloat16)

    in_maps = []
    for c in range(NCORES):
        g = c // 2
        wq_c = wq[128 * c:128 * (c + 1)].reshape(2, HD, DIM)[:, perm, :].reshape(128, DIM)
        wk_g = wk[HD * g:HD * (g + 1)][perm]
        wv_g = wv[HD * g:HD * (g + 1)]
        wkv_c = np.concatenate([wv_g, wk_g], axis=0)        # v rows 0:64, k rows 64:128
        wo_c = np.ascontiguousarray(wo[:, 128 * c:128 * (c + 1)].T).astype(
            ml_dtypes.bfloat16)                              # [128(j), 1024(o)]
        in_maps.append({
            "xT": xT,
            "wq_l": fold(wq_c),
            "wkv_l": fold(wkv_c),
            "wo_l": wo_c,
            "cos4": cos4,
            "sin4": sin4,
            "mask": mask,
            "ones32": ones32,
            "ident": ident,
            "sel2": sel2,
        })
    return in_maps


def kernel(x, freqs_cos, freqs_sin, wq, wk, wv, wo, _trace=False, _trace_kwargs=None):
    nc = _build()
    in_maps = _host_inputs(x, freqs_cos, freqs_sin, wq, wk, wv, wo)
    kw = {}
    if _trace:
        kw.update(trace=True, **(_trace_kwargs or {}))
    res = run_bass_kernel_spmd(nc, in_maps, core_ids=list(range(NCORES)), **kw)
    acc = np.zeros((DIM, SEQ), np.float32)
    for c in range(NCORES):
        acc += res.results[c]["out"]
    out = np.ascontiguousarray(acc.T).reshape(1, SEQ, DIM)
    if _trace:
        kernel._last_results = res
    return out


# revision 9
# speedup vs baseline: 1.2989x; 1.0464x over previous
"""GQA causal attention (RoPE) on 8 Trainium2 NeuronCores.

Sharding (tensor-parallel over heads, per the hint):
  core c owns q-heads {2c, 2c+1} and kv-head c//2.
  Each core computes its 2 heads' attention over the full sequence and a
  partial output projection out_c.T = wo[:, 128c:128c+128] @ att_c  (shape
  [1024, 4096]); the final all-reduce over cores is the host-side unshard.

Device-side per core (v4 — fused + software-pipelined over 512-row chunks):
  loop n: proj(n) -> rope(n) -> v^T(n) -> normalize+wo(n-1) -> attention(n).
  The chunk-(n-1) normalization (DVE reciprocal chain) overlaps chunk-n PE
  work, so the PE stream never waits on it and stays HAM-warm.

  - All matmuls bf16 (1 cycle/row; fp32r streams at ~2 cycles/row), fp32 PSUM.
  - RoPE in fp32 from PSUM: host-side even/odd permutation folded into wq/wk,
    sign-folded sin tile, partition-block swap via SBUF->SBUF DMA; bf16 out.
  - scores^T blocks [sk=128, sq=512]: single K=64 matmuls; the two heads hit
    disjoint PE row groups (k duplicated in both halves) -> concurrent.
  - Causal masks accumulated on the PE (identity matmul, -1e6 bf16 additive
    masks) before a grouped PSUM exp on ScalarE.
  - AV with ones-augmented V^T so softmax denominators fall out of the same
    matmul; V^T via PE transpose-mode, evacuated by ScalarE copies.
  - Normalization: raw AV staged to SBUF (f32r), both heads' denominators
    merged into one K=2 broadcast matmul, one reciprocal per chunk,
    per-column scale, then the 8 wo matmuls for the chunk.
"""
import numpy as np
import ml_dtypes
from contextlib import ExitStack

import concourse.bacc as bacc
import concourse.tile as tile
import concourse.mybir as mybir
from concourse.bass_utils import run_bass_kernel_spmd

DIM = 1024
N_HEADS = 16
N_KV = 4
HD = 64
SEQ = 4096
NCORES = 8

SQ = 512            # query-chunk (free dim of score blocks)
SK = 128            # key-chunk (partition dim of score blocks)
NQ = SEQ // SQ      # 8
NR = DIM // 128     # 8 contraction chunks for projections
NJ = SEQ // SK      # 32 key chunks
GROUP = 2           # score blocks per exp instruction ([128, 1024] PSUM)
MASKVAL = -1.0e6

f32 = mybir.dt.float32
f32r = mybir.dt.float32r
bf16 = mybir.dt.bfloat16
FT = mybir.ActivationFunctionType

_CACHE = {}


def _emit(nc):
    xT = nc.dram_tensor("xT", [DIM, SEQ], bf16, kind="ExternalInput").ap()
    wq_l = nc.dram_tensor("wq_l", [128, DIM], bf16, kind="ExternalInput").ap()
    wkv_l = nc.dram_tensor("wkv_l", [128, DIM], bf16, kind="ExternalInput").ap()
    wo_l = nc.dram_tensor("wo_l", [128, DIM], bf16, kind="ExternalInput").ap()
    cos4_d = nc.dram_tensor("cos4", [128, SEQ], f32, kind="ExternalInput").ap()
    sin4_d = nc.dram_tensor("sin4", [128, SEQ], f32, kind="ExternalInput").ap()
    mask_d = nc.dram_tensor("mask", [128, 4 * SQ], bf16, kind="ExternalInput").ap()
    on_d = nc.dram_tensor("ones32", [128, NJ], bf16, kind="ExternalInput").ap()
    id_d = nc.dram_tensor("ident", [128, 128], bf16, kind="ExternalInput").ap()
    sel_d = nc.dram_tensor("sel2", [128, 128], f32r, kind="ExternalInput").ap()
    out_d = nc.dram_tensor("out", [DIM, SEQ], f32, kind="ExternalOutput").ap()

    with tile.TileContext(nc) as tc, ExitStack() as ctx:
        const = ctx.enter_context(tc.tile_pool(name="const", bufs=1))
        main = ctx.enter_context(tc.tile_pool(name="main", bufs=1))

        wq_sb = const.tile([128, DIM], bf16)
        wkv_sb = const.tile([128, DIM], bf16)
        wo_sb = const.tile([128, DIM], bf16)
        cos_sb = const.tile([128, SEQ], f32)
        sin_sb = const.tile([128, SEQ], f32)
        msk_sb = const.tile([128, 4 * SQ], bf16)
        id_sb = const.tile([128, 128], bf16)
        sel_sb = const.tile([128, 128], f32r)
        nc.sync.dma_start(wq_sb[:], wq_l[:])
        nc.sync.dma_start(wkv_sb[:], wkv_l[:])
        nc.sync.dma_start(wo_sb[:], wo_l[:])
        nc.sync.dma_start(id_sb[:], id_d[:])
        nc.sync.dma_start(sel_sb[:], sel_d[:])
        nc.sync.dma_start(msk_sb[:], mask_d[:])

        qrot = main.tile([128, SEQ], bf16)      # 2 heads d-major (rope'd)
        krot = main.tile([128, SEQ], bf16)      # k duplicated in both halves
        v_sb = main.tile([HD, SEQ], bf16)       # v d-major
        vt = main.tile([128, NJ, 128], bf16)    # v^T + ones column (aligned slots)
        attS = main.tile([128, SEQ], bf16)      # stacked normalized att (j-major)
        att1 = main.tile([HD, SEQ], bf16)       # head-1 att staging (lanes 0-63)
        au0 = main.tile([HD + 1, SEQ], f32r)    # raw AV + denom staging, head 0
        au1 = main.tile([HD + 1, SEQ], f32r)    # head 1
        d2 = main.tile([66, SEQ], f32r)         # merged denoms (rows 64, 65)

        # ones column of vt via one DMA
        nc.sync.dma_start(vt[:, :, HD:HD + 1], on_d[:])

        with (
            tc.tile_pool(name="xp", bufs=4) as xp,
            tc.tile_pool(name="pp", bufs=1, space="PSUM") as pp,
            tc.tile_pool(name="rp", bufs=2) as rp,
            tc.tile_pool(name="sp", bufs=2, space="PSUM") as sp,
            tc.tile_pool(name="ap", bufs=1, space="PSUM") as ap,
            tc.tile_pool(name="ep", bufs=4) as ep,
            tc.tile_pool(name="rbp", bufs=2) as rbp,
            tc.tile_pool(name="op", bufs=4) as op,
        ):
            def endgame(k):
                sk0 = k * SQ
                nc.sync.dma_start(d2[64:65, sk0:sk0 + SQ], au0[HD:HD + 1, sk0:sk0 + SQ])
                nc.sync.dma_start(d2[65:66, sk0:sk0 + SQ], au1[HD:HD + 1, sk0:sk0 + SQ])
                bc = sp.tile([128, SQ], f32, tag="sc", name=f"bc_{k}")
                nc.tensor.matmul(bc[:], sel_sb[64:66, :], d2[64:66, sk0:sk0 + SQ],
                                 start=True, stop=True)
                rb = rbp.tile([128, SQ], f32, tag="rb")
                nc.vector.reciprocal(rb[:], bc[:])
                rb1 = rbp.tile([HD, SQ], f32, tag="rb1")
                nc.sync.dma_start(rb1[:], rb[64:64 + HD, :])
                nc.vector.tensor_mul(attS[0:HD, sk0:sk0 + SQ],
                                     au0[0:HD, sk0:sk0 + SQ].bitcast(f32), rb[0:HD, :])
                nc.vector.tensor_mul(att1[:, sk0:sk0 + SQ],
                                     au1[0:HD, sk0:sk0 + SQ].bitcast(f32), rb1[:])
                nc.sync.dma_start(attS[64:128, sk0:sk0 + SQ], att1[:, sk0:sk0 + SQ])
                for m in range(8):
                    pw = sp.tile([128, SQ], f32, tag="sc", name=f"pw_{k}_{m}")
                    nc.tensor.matmul(pw[:], wo_sb[:, 128 * m:128 * (m + 1)],
                                     attS[:, sk0:sk0 + SQ], start=True, stop=True)
                    ot = op.tile([128, SQ], f32)
                    nc.vector.tensor_copy(ot[:], pw[:])
                    nc.sync.dma_start(out_d[128 * m:128 * (m + 1), sk0:sk0 + SQ], ot[:])

            for n in range(NQ):
                s0 = n * SQ
                # trig chunks for this iteration (overlap with projections)
                nc.sync.dma_start(cos_sb[:, s0:s0 + SQ], cos4_d[:, s0:s0 + SQ])
                nc.sync.dma_start(sin_sb[:, s0:s0 + SQ], sin4_d[:, s0:s0 + SQ])
                # ---- projections ----
                pq = pp.tile([128, SQ], f32, tag="pq")
                pkv = pp.tile([128, SQ], f32, tag="pkv")
                for r in range(NR):
                    xt = xp.tile([128, SQ], bf16)
                    nc.sync.dma_start(xt[:], xT[128 * r:128 * (r + 1), s0:s0 + SQ])
                    nc.tensor.matmul(pq[:], wq_sb[:, 128 * r:128 * (r + 1)], xt[:],
                                     start=(r == 0), stop=(r == NR - 1))
                    nc.tensor.matmul(pkv[:], wkv_sb[:, 128 * r:128 * (r + 1)], xt[:],
                                     start=(r == 0), stop=(r == NR - 1))
                # ---- rope q ----
                a_t = rp.tile([128, SQ], f32, tag="ta")
                c_t = rp.tile([128, SQ], f32, tag="tc")
                b_t = rp.tile([128, SQ], f32, tag="tb")
                nc.vector.tensor_mul(a_t[:], pq[:], cos_sb[:, s0:s0 + SQ])
                nc.vector.tensor_mul(c_t[:], pq[:], sin_sb[:, s0:s0 + SQ])
                nc.sync.dma_start(b_t[0:32, :], c_t[32:64, :])
                nc.sync.dma_start(b_t[32:64, :], c_t[0:32, :])
                nc.sync.dma_start(b_t[64:96, :], c_t[96:128, :])
                nc.sync.dma_start(b_t[96:128, :], c_t[64:96, :])
                nc.vector.tensor_add(qrot[:, s0:s0 + SQ], a_t[:], b_t[:])
                # ---- rope k (rows 64:128; v occupies rows 0:64) ----
                ak = rp.tile([128, SQ], f32, tag="ta")
                ck = rp.tile([128, SQ], f32, tag="tc")
                bk = rp.tile([128, SQ], f32, tag="tb")
                nc.vector.tensor_mul(ak[64:128, :], pkv[64:128, :], cos_sb[64:128, s0:s0 + SQ])
                nc.vector.tensor_mul(ck[64:128, :], pkv[64:128, :], sin_sb[64:128, s0:s0 + SQ])
                nc.sync.dma_start(bk[64:96, :], ck[96:128, :])
                nc.sync.dma_start(bk[96:128, :], ck[64:96, :])
                nc.vector.tensor_add(krot[64:128, s0:s0 + SQ], ak[64:128, :], bk[64:128, :])
                nc.sync.dma_start(krot[0:64, s0:s0 + SQ], krot[64:128, s0:s0 + SQ])
                # ---- v -> bf16, PE transpose into vt (ScalarE evacuates) ----
                nc.vector.tensor_copy(v_sb[:, s0:s0 + SQ], pkv[0:64, :])
                for j in range(4 * n, 4 * n + 4):
                    pt = sp.tile([SK, HD], bf16, tag="sc", name=f"pt_{j}")
                    nc.tensor.transpose(pt[:], v_sb[:, SK * j:SK * (j + 1)],
                                        id_sb[0:HD, 0:HD])
                    nc.vector.tensor_copy(vt[:, j, 0:HD], pt[:])

                # ---- previous chunk's normalization + wo (overlaps PE work) ----
                if n > 0:
                    endgame(n - 1)

                # ---- attention ----
                nsk = 4 * (n + 1)
                av = [ap.tile([HD + 1, SQ], f32, tag=f"av{h}", name=f"av{h}_{n}")
                      for h in (0, 1)]
                blocks = [(j, h) for j in range(nsk) for h in (0, 1)]
                groups = [blocks[g0:g0 + GROUP] for g0 in range(0, len(blocks), GROUP)]
                pend = []   # (grp, et) awaiting AV emission (lag 2)

                def flush_av(n_=n):
                    grp_, et_ = pend.pop(0)
                    nsk_ = 4 * (n_ + 1)
                    for i_, (j_, h_) in enumerate(grp_):
                        nc.tensor.matmul(
                            av[h_][:], vt[:, j_, 0:HD + 1],
                            et_[:, i_ * SQ:(i_ + 1) * SQ],
                            start=(j_ == 0), stop=(j_ == nsk_ - 1),
                        )

                for grp in groups:
                    sc = sp.tile([128, GROUP * SQ], f32, tag="sc")
                    for i, (j, h) in enumerate(grp):
                        o = i * SQ
                        delta = SK * j - s0
                        diag = delta >= 0
                        nc.tensor.matmul(
                            sc[:, o:o + SQ],
                            krot[64 * h:64 * h + 64, SK * j:SK * (j + 1)],
                            qrot[64 * h:64 * h + 64, s0:s0 + SQ],
                            start=True, stop=not diag,
                        )
                        if diag:
                            w = min(SQ, delta + SK)
                            db = (delta // SK) * SQ
                            nc.tensor.matmul(sc[:, o:o + w], id_sb[:],
                                             msk_sb[:, db:db + w],
                                             start=False, stop=True)
                    ew = len(grp) * SQ
                    et = ep.tile([128, GROUP * SQ], bf16, tag="et")
                    nc.scalar.activation(et[:, 0:ew], sc[:, 0:ew], FT.Exp, scale=0.125)
                    pend.append((grp, et))
                    if len(pend) > 2:
                        flush_av()
                while pend:
                    flush_av()
                # ---- stage raw AV, free banks ----
                nc.vector.tensor_copy(au0[:, s0:s0 + SQ], av[0][:])
                nc.vector.tensor_copy(au1[:, s0:s0 + SQ], av[1][:])

            endgame(NQ - 1)


def _build():
    if "nc" in _CACHE:
        return _CACHE["nc"]
    nc = bacc.Bacc("TRN2", target_bir_lowering=False, debug=False, num_devices=NCORES)
    _emit(nc)
    nc.compile()
    _CACHE["nc"] = nc
    return nc


def _host_inputs(x, freqs_cos, freqs_sin, wq, wk, wv, wo):
    x = np.asarray(x, np.float32)
    freqs_cos = np.asarray(freqs_cos, np.float32)
    freqs_sin = np.asarray(freqs_sin, np.float32)
    wq = np.asarray(wq, np.float32)
    wk = np.asarray(wk, np.float32)
    wv = np.asarray(wv, np.float32)
    wo = np.asarray(wo, np.float32)

    xT = np.ascontiguousarray(x[0].T).astype(ml_dtypes.bfloat16)   # [1024, 4096]
    cosT = freqs_cos.T                                             # [32, 4096]
    sinT = freqs_sin.T
    cos4 = np.ascontiguousarray(np.tile(cosT, (4, 1)))             # [128, 4096]
    sin4 = np.ascontiguousarray(
        np.concatenate([sinT, -sinT, sinT, -sinT], axis=0))

    # diagonal-block causal masks for delta in {0,128,256,384}
    p = np.arange(SK)[:, None]
    f = np.arange(SQ)[None, :]
    mask = np.concatenate(
        [np.where(SK * d + p <= f, 0.0, MASKVAL) for d in range(4)],
        axis=1).astype(ml_dtypes.bfloat16)                         # [128, 2048]

    ones32 = np.ones((128, NJ), dtype=ml_dtypes.bfloat16)
    ident = np.eye(128, dtype=ml_dtypes.bfloat16)
    sel2 = np.zeros((128, 128), dtype=np.float32)
    sel2[64, 0:64] = 1.0
    sel2[65, 64:128] = 1.0

    perm = np.concatenate([np.arange(0, HD, 2), np.arange(1, HD, 2)])

    def fold(w):  # [128(m), 1024(d)] -> lhsT layout [128(p), 8r*128+m]
        return np.ascontiguousarray(
            w.reshape(128, NR, 128).transpose(2, 1, 0).reshape(128, DIM)
        ).astype(ml_dtypes.bfloat16)

    in_maps = []
    for c in range(NCORES):
        g = c // 2
        wq_c = wq[128 * c:128 * (c + 1)].reshape(2, HD, DIM)[:, perm, :].reshape(128, DIM)
        wk_g = wk[HD * g:HD * (g + 1)][perm]
        wv_g = wv[HD * g:HD * (g + 1)]
        wkv_c = np.concatenate([wv_g, wk_g], axis=0)        # v rows 0:64, k rows 64:128
        wo_c = np.ascontiguousarray(wo[:, 128 * c:128 * (c + 1)].T).astype(
            ml_dtypes.bfloat16)                              # [128(j), 1024(o)]
        in_maps.append({
            "xT": xT,
            "wq_l": fold(wq_c),
            "wkv_l": fold(wkv_c),
            "wo_l": wo_c,
            "cos4": cos4,
            "sin4": sin4,
            "mask": mask,
            "ones32": ones32,
            "ident": ident,
            "sel2": sel2,
        })
    return in_maps


def kernel(x, freqs_cos, freqs_sin, wq, wk, wv, wo, _trace=False, _trace_kwargs=None):
    nc = _build()
    in_maps = _host_inputs(x, freqs_cos, freqs_sin, wq, wk, wv, wo)
    kw = {}
    if _trace:
        kw.update(trace=True, **(_trace_kwargs or {}))
    res = run_bass_kernel_spmd(nc, in_maps, core_ids=list(range(NCORES)), **kw)
    acc = np.zeros((DIM, SEQ), np.float32)
    for c in range(NCORES):
        acc += res.results[c]["out"]
    out = np.ascontiguousarray(acc.T).reshape(1, SEQ, DIM)
    if _trace:
        kernel._last_results = res
    return out
